# revision 8
# baseline (speedup 1.0000x reference)
"""v5.3: bf16 attention on 8 trn2 NeuronCores, host-transposed inputs.

Core c handles batch c//2, query half c%2, loading the batch's full K and V.
Host pre-casts q/k/v to bf16 and pre-transposes them to [H, L] layout so all
device DMA loads are linear and the PE never transposes the 1024-dim inputs.
All big loads go through one SWDGE queue in exact consumption order (qT,
then kT/vT per 512-range) so compute streams right behind the DMA. The
1/sqrt(64) score scale is folded into Wq/bq on the host. Projection bias
adds run on the DVE; the Act engine only does exp (one [128,1024] activation
per key-chunk). Scores for chunk kc+1 issue before attnV of chunk kc so the
PE never waits on the Act engine. Softmax free-dim is never reduced: a
ones-column in the attnV stationary yields row-sums; normalization happens
in the output epilogue.
"""

import sys

if "/opt/trn_rl_repo" not in sys.path:
    sys.path.insert(0, "/opt/trn_rl_repo")

import numpy as np

N, L, H, D = 4, 2048, 1024, 64
QSH = L // 2
NCORES = 8
HC = H // 128
NRNG = L // 512
NKC = L // 128


def build_bass():
    import concourse.bass as bass
    import concourse.mybir as mybir
    from concourse import bacc
    from concourse.masks import make_identity
    from concourse.tile import TileContext

    f32 = mybir.dt.float32
    bf16 = mybir.dt.bfloat16
    AF = mybir.ActivationFunctionType

    nc = bacc.Bacc("TRN2", target_bir_lowering=False, debug=False)
    # host-transposed, bf16: qT [H, QSH], kT/vT [H, L]
    qT_d = nc.dram_tensor("qT", [H, QSH], bf16, kind="ExternalInput").ap()
    kT_d = nc.dram_tensor("kT", [H, L], bf16, kind="ExternalInput").ap()
    vT_d = nc.dram_tensor("vT", [H, L], bf16, kind="ExternalInput").ap()
    wq_d = nc.dram_tensor("wq", [H, D], bf16, kind="ExternalInput").ap()
    wk_d = nc.dram_tensor("wk", [H, D], bf16, kind="ExternalInput").ap()
    wv_d = nc.dram_tensor("wv", [H, D], bf16, kind="ExternalInput").ap()
    bq_d = nc.dram_tensor("bq8", [D, 1], f32, kind="ExternalInput").ap()
    bk_d = nc.dram_tensor("bk", [D, 1], f32, kind="ExternalInput").ap()
    bv_d = nc.dram_tensor("bv", [D, 1], f32, kind="ExternalInput").ap()
    out_d = nc.dram_tensor("out", [QSH, D], f32, kind="ExternalOutput").ap()

    with TileContext(nc) as tc:
        with (
            tc.tile_pool(name="const", bufs=1) as const_pool,
            tc.tile_pool(name="w", bufs=1) as w_pool,
            tc.tile_pool(name="qT", bufs=1) as qT_pool,
            tc.tile_pool(name="kt", bufs=4) as kt_pool,
            tc.tile_pool(name="vt", bufs=4) as vt_pool,
            tc.tile_pool(name="proj", bufs=1) as proj_pool,
            tc.tile_pool(name="vp", bufs=1) as vp_pool,
            tc.tile_pool(name="exp", bufs=4) as exp_pool,
            tc.tile_pool(name="fin", bufs=1) as fin_pool,
            tc.tile_pool(name="pj", bufs=1, space="PSUM") as pj_psum,
            tc.tile_pool(name="sc", bufs=2, space="PSUM") as sc_psum,
            tc.tile_pool(name="psv", bufs=1, space="PSUM") as psv_psum,
            tc.tile_pool(name="acc", bufs=1, space="PSUM") as acc_psum,
        ):
            # tiny loads on sync: biases + weights
            bq_sb = const_pool.tile([D, 1], f32, tag="bq")
            bk_sb = const_pool.tile([D, 1], f32, tag="bk")
            bv_sb = const_pool.tile([D, 1], f32, tag="bv")
            nc.sync.dma_start(out=bq_sb[:], in_=bq_d[:])
            nc.sync.dma_start(out=bk_sb[:], in_=bk_d[:])
            nc.sync.dma_start(out=bv_sb[:], in_=bv_d[:])
            w_sb = {}
            for name, wd in (("wq", wq_d), ("wk", wk_d), ("wv", wv_d)):
                t = w_pool.tile([128, HC * D], bf16, tag=name, name=name)
                nc.sync.dma_start(
                    out=t[:].rearrange("p (c d) -> p c d", c=HC),
                    in_=wd.rearrange("(c p) d -> p c d", p=128),
                )
                w_sb[name] = t

            # big loads: one SWDGE queue, in exact consumption order
            qT = qT_pool.tile([128, HC * QSH], bf16)
            nc.gpsimd.dma_start(
                out=qT[:].rearrange("p (c l) -> p c l", c=HC),
                in_=qT_d.rearrange("(c p) l -> p c l", p=128),
            )
            kts, vts = [], []
            for rng in range(NRNG):
                kt = kt_pool.tile([128, HC * 512], bf16, tag="kt",
                                  name=f"kT{rng}")
                nc.gpsimd.dma_start(
                    out=kt[:].rearrange("p (c l) -> p c l", c=HC),
                    in_=kT_d.rearrange("(c p) l -> p c l", p=128)[
                        :, :, rng * 512 : (rng + 1) * 512
                    ],
                )
                kts.append(kt)
                vt = vt_pool.tile([128, HC * 512], bf16, tag="vt",
                                  name=f"vT{rng}")
                nc.gpsimd.dma_start(
                    out=vt[:].rearrange("p (c l) -> p c l", c=HC),
                    in_=vT_d.rearrange("(c p) l -> p c l", p=128)[
                        :, :, rng * 512 : (rng + 1) * 512
                    ],
                )
                vts.append(vt)

            identb = const_pool.tile([128, 128], bf16)
            make_identity(nc, identb[:])
            identf = const_pool.tile([128, 128], f32, tag="identf")
            make_identity(nc, identf[:])

            vp = vp_pool.tile([128, NKC * 65], bf16, tag="vp")
            nc.vector.memset(
                vp[:].rearrange("p (c e) -> p c e", e=65)[:, :, 64:65], 1.0
            )

            # ---- Q projection ----
            qprojT = proj_pool.tile([D, QSH], bf16, tag="qprojT")
            for qn in range(QSH // 512):
                ps = pj_psum.tile([D, 512], f32, tag="pj", name="pjq")
                for hc in range(HC):
                    nc.tensor.matmul(
                        ps[:],
                        w_sb["wq"][:, hc * D : (hc + 1) * D],
                        qT[:, hc * QSH + qn * 512 : hc * QSH + (qn + 1) * 512],
                        start=(hc == 0), stop=(hc == HC - 1),
                    )
                nc.vector.tensor_scalar_add(
                    qprojT[:, qn * 512 : (qn + 1) * 512], ps[:], bq_sb[:]
                )

            def kproj(rng):
                ps = pj_psum.tile([D, 512], f32, tag="pj", name=f"pjk{rng}")
                for hc in range(HC):
                    nc.tensor.matmul(
                        ps[:], w_sb["wk"][:, hc * D : (hc + 1) * D],
                        kts[rng][:, hc * 512 : (hc + 1) * 512],
                        start=(hc == 0), stop=(hc == HC - 1),
                    )
                ks = proj_pool.tile([D, 512], bf16, tag=f"ks{rng % 2}",
                                    name=f"ks{rng}")
                nc.vector.tensor_scalar_add(ks[:], ps[:], bk_sb[:])
                return ks

            def vproj_trans(rng):
                ps = pj_psum.tile([D, 512], f32, tag="pj", name=f"pjv{rng}")
                for hc in range(HC):
                    nc.tensor.matmul(
                        ps[:], w_sb["wv"][:, hc * D : (hc + 1) * D],
                        vts[rng][:, hc * 512 : (hc + 1) * 512],
                        start=(hc == 0), stop=(hc == HC - 1),
                    )
                vs = proj_pool.tile([D, 512], bf16, tag=f"vs{rng % 2}",
                                    name=f"vs{rng}")
                nc.vector.tensor_scalar_add(vs[:], ps[:], bv_sb[:])
                psv = psv_psum.tile([128, 512], bf16, tag="psv", name="psv")
                for s in range(4):
                    nc.tensor.transpose(
                        psv[:, s * 128 : s * 128 + D],
                        vs[:, s * 128 : (s + 1) * 128],
                        identb[0:D, 0:D],
                    )
                for s in range(4):
                    kc = rng * 4 + s
                    nc.vector.tensor_copy(
                        vp[:, kc * 65 : kc * 65 + 64],
                        psv[:, s * 128 : s * 128 + D],
                    )

            outT_ps = acc_psum.tile([65, QSH], f32)
            ks_tiles = {}

            def issue_sc(kc):
                """scores for key-chunk kc + one exp activation; returns e."""
                rng, s = kc // 4, kc % 4
                ks = ks_tiles[rng]
                sct = sc_psum.tile([128, QSH], f32, tag="sc")
                for qn in range(QSH // 512):
                    nc.tensor.matmul(
                        sct[:, qn * 512 : (qn + 1) * 512],
                        ks[:, s * 128 : (s + 1) * 128],
                        qprojT[:, qn * 512 : (qn + 1) * 512],
                        start=True, stop=True,
                    )
                e = exp_pool.tile([128, QSH], bf16, tag="exp")
                nc.scalar.activation(e[:], sct[:], AF.Exp)
                return e

            def issue_av(kc, e):
                for qn in range(QSH // 512):
                    nc.tensor.matmul(
                        outT_ps[:, qn * 512 : (qn + 1) * 512],
                        vp[:, kc * 65 : (kc + 1) * 65],
                        e[:, qn * 512 : (qn + 1) * 512],
                        start=(kc == 0), stop=(kc == NKC - 1),
                        skip_group_check=True,
                    )

            # prime two ranges of K/V projection work
            ks_tiles[0] = kproj(0)
            vproj_trans(0)
            ks_tiles[1] = kproj(1)
            vproj_trans(1)
            e_next = issue_sc(0)
            for kc in range(NKC):
                e_cur = e_next
                if kc % 4 == 2 and kc // 4 + 2 < NRNG:
                    rng = kc // 4 + 2
                    ks_tiles[rng] = kproj(rng)
                    vproj_trans(rng)
                if kc + 1 < NKC:
                    e_next = issue_sc(kc + 1)
                issue_av(kc, e_cur)

            # ---- finalize ----
            outT_sb = fin_pool.tile([65, QSH], f32, tag="outT")
            nc.vector.tensor_copy(outT_sb[:], outT_ps[:])
            out_sb = fin_pool.tile([128, 8 * D], f32, tag="out")
            for qc in range(QSH // 128):
                ps = pj_psum.tile([128, 128], f32, tag="pj", name="pjf")
                nc.tensor.transpose(
                    ps[:, 0:65],
                    outT_sb[:, qc * 128 : (qc + 1) * 128],
                    identf[0:65, 0:65],
                )
                recip = fin_pool.tile([128, 1], f32, tag="recip")
                nc.vector.reciprocal(recip[:], ps[:, 64:65])
                nc.vector.tensor_scalar_mul(
                    out_sb[:, qc * D : (qc + 1) * D], ps[:, 0:D], recip[:]
                )
            nc.sync.dma_start(
                out=out_d.rearrange("(j p) d -> p j d", p=128),
                in_=out_sb[:].rearrange("p (j d) -> p j d", j=8),
            )

    nc.compile()
    return nc


_NC_CACHE = None


def _get_nc():
    global _NC_CACHE
    if _NC_CACHE is None:
        _NC_CACHE = build_bass()
    return _NC_CACHE


def _make_in_maps(inputs):
    import ml_dtypes

    bf = ml_dtypes.bfloat16
    # [N, L, H] -> [N, H, L] transposed bf16, contiguous
    qt = np.ascontiguousarray(
        np.asarray(inputs["query"], np.float32).astype(bf).transpose(0, 2, 1)
    )
    kt = np.ascontiguousarray(
        np.asarray(inputs["key"], np.float32).astype(bf).transpose(0, 2, 1)
    )
    vt = np.ascontiguousarray(
        np.asarray(inputs["value"], np.float32).astype(bf).transpose(0, 2, 1)
    )
    # fold the 1/sqrt(D) score scale into Wq and bq
    wq = np.ascontiguousarray(
        (np.asarray(inputs["Wq"], np.float32) / 8.0).astype(bf)
    )
    wk = np.ascontiguousarray(np.asarray(inputs["Wk"], np.float32).astype(bf))
    wv = np.ascontiguousarray(np.asarray(inputs["Wv"], np.float32).astype(bf))
    bq8 = (np.asarray(inputs["bq"], np.float32) / 8.0).reshape(D, 1)
    bk = np.asarray(inputs["bk"], np.float32).reshape(D, 1).copy()
    bv = np.asarray(inputs["bv"], np.float32).reshape(D, 1).copy()
    in_maps = []
    for c in range(NCORES):
        b, half = divmod(c, 2)
        in_maps.append(
            {
                "qT": np.ascontiguousarray(
                    qt[b, :, half * QSH : (half + 1) * QSH]
                ),
                "kT": kt[b],
                "vT": vt[b],
                "wq": wq,
                "wk": wk,
                "wv": wv,
                "bq8": bq8,
                "bk": bk,
                "bv": bv,
            }
        )
    return in_maps


def kernel(query, key, value, Wq, bq, Wk, bk, Wv, bv):
    from concourse.bass_utils import run_bass_kernel_spmd

    in_maps = _make_in_maps(
        dict(query=query, key=key, value=value, Wq=Wq, bq=bq, Wk=Wk, bk=bk,
             Wv=Wv, bv=bv)
    )
    nc = _get_nc()
    try:
        res = run_bass_kernel_spmd(nc, in_maps, list(range(NCORES)))
    except Exception:
        res = run_bass_kernel_spmd(nc, in_maps, list(range(NCORES)))
    out = np.empty((N, L, D), np.float32)
    for c in range(NCORES):
        b, half = divmod(c, 2)
        out[b, half * QSH : (half + 1) * QSH] = res.results[c]["out"]
    return out


# revision 9
# speedup vs baseline: 1.1202x; 1.1202x over previous
"""v5.3: bf16 attention on 8 trn2 NeuronCores, host-transposed inputs.

Core c handles batch c//2, query half c%2, loading the batch's full K and V.
Host pre-casts q/k/v to bf16 and pre-transposes them to [H, L] layout so all
device DMA loads are linear and the PE never transposes the 1024-dim inputs.
All big loads go through one SWDGE queue in exact consumption order (qT,
then kT/vT per 512-range) so compute streams right behind the DMA. The
1/sqrt(64) score scale is folded into Wq/bq on the host. Projection bias
adds run on the DVE; the Act engine only does exp (one [128,1024] activation
per key-chunk). Scores for chunk kc+1 issue before attnV of chunk kc so the
PE never waits on the Act engine. Softmax free-dim is never reduced: a
ones-column in the attnV stationary yields row-sums; normalization happens
in the output epilogue.
"""

import sys

if "/opt/trn_rl_repo" not in sys.path:
    sys.path.insert(0, "/opt/trn_rl_repo")

import numpy as np

N, L, H, D = 4, 2048, 1024, 64
QSH = L // 2
NCORES = 8
HC = H // 128
NRNG = L // 512
NKC = L // 128


def build_bass():
    import concourse.bass as bass
    import concourse.mybir as mybir
    from concourse import bacc
    from concourse.masks import make_identity
    from concourse.tile import TileContext

    f32 = mybir.dt.float32
    bf16 = mybir.dt.bfloat16
    AF = mybir.ActivationFunctionType

    nc = bacc.Bacc("TRN2", target_bir_lowering=False, debug=False)
    # host-transposed, bf16: qT [H, QSH], kT/vT [H, L]
    qT_d = nc.dram_tensor("qT", [H, QSH], bf16, kind="ExternalInput").ap()
    kT_d = nc.dram_tensor("kT", [H, L], bf16, kind="ExternalInput").ap()
    vT_d = nc.dram_tensor("vT", [H, L], bf16, kind="ExternalInput").ap()
    wq_d = nc.dram_tensor("wq", [H, D], bf16, kind="ExternalInput").ap()
    wk_d = nc.dram_tensor("wk", [H, D], bf16, kind="ExternalInput").ap()
    wv_d = nc.dram_tensor("wv", [H, D], bf16, kind="ExternalInput").ap()
    bq_d = nc.dram_tensor("bq8", [D, 1], f32, kind="ExternalInput").ap()
    bk_d = nc.dram_tensor("bk", [D, 1], f32, kind="ExternalInput").ap()
    bv_d = nc.dram_tensor("bv", [D, 1], f32, kind="ExternalInput").ap()
    out_d = nc.dram_tensor("out", [QSH, D], f32, kind="ExternalOutput").ap()

    with TileContext(nc) as tc:
        with (
            tc.tile_pool(name="const", bufs=1) as const_pool,
            tc.tile_pool(name="w", bufs=1) as w_pool,
            tc.tile_pool(name="qT", bufs=1) as qT_pool,
            tc.tile_pool(name="kt", bufs=4) as kt_pool,
            tc.tile_pool(name="vt", bufs=4) as vt_pool,
            tc.tile_pool(name="proj", bufs=1) as proj_pool,
            tc.tile_pool(name="vp", bufs=1) as vp_pool,
            tc.tile_pool(name="exp", bufs=4) as exp_pool,
            tc.tile_pool(name="fin", bufs=1) as fin_pool,
            tc.tile_pool(name="pj", bufs=1, space="PSUM") as pj_psum,
            tc.tile_pool(name="sc", bufs=4, space="PSUM") as sc_psum,
            tc.tile_pool(name="psv", bufs=1, space="PSUM") as psv_psum,
            tc.tile_pool(name="acc", bufs=1, space="PSUM") as acc_psum,
        ):
            # tiny loads on sync: biases + weights
            bq_sb = const_pool.tile([D, 1], f32, tag="bq")
            bk_sb = const_pool.tile([D, 1], f32, tag="bk")
            bv_sb = const_pool.tile([D, 1], f32, tag="bv")
            nc.sync.dma_start(out=bq_sb[:], in_=bq_d[:])
            nc.sync.dma_start(out=bk_sb[:], in_=bk_d[:])
            nc.sync.dma_start(out=bv_sb[:], in_=bv_d[:])
            w_sb = {}
            for name, wd in (("wq", wq_d), ("wk", wk_d), ("wv", wv_d)):
                t = w_pool.tile([128, HC * D], bf16, tag=name, name=name)
                nc.gpsimd.dma_start(
                    out=t[:].rearrange("p (c d) -> p c d", c=HC),
                    in_=wd.rearrange("(c p) d -> p c d", p=128),
                )
                w_sb[name] = t

            # big loads: one SWDGE queue, in exact consumption order
            qT = qT_pool.tile([128, HC * QSH], bf16)
            nc.gpsimd.dma_start(
                out=qT[:].rearrange("p (c l) -> p c l", c=HC),
                in_=qT_d.rearrange("(c p) l -> p c l", p=128),
            )
            kts, vts = [], []
            for rng in range(NRNG):
                kt = kt_pool.tile([128, HC * 512], bf16, tag="kt",
                                  name=f"kT{rng}")
                nc.gpsimd.dma_start(
                    out=kt[:].rearrange("p (c l) -> p c l", c=HC),
                    in_=kT_d.rearrange("(c p) l -> p c l", p=128)[
                        :, :, rng * 512 : (rng + 1) * 512
                    ],
                )
                kts.append(kt)
                vt = vt_pool.tile([128, HC * 512], bf16, tag="vt",
                                  name=f"vT{rng}")
                nc.gpsimd.dma_start(
                    out=vt[:].rearrange("p (c l) -> p c l", c=HC),
                    in_=vT_d.rearrange("(c p) l -> p c l", p=128)[
                        :, :, rng * 512 : (rng + 1) * 512
                    ],
                )
                vts.append(vt)

            identb = const_pool.tile([128, 128], bf16)
            make_identity(nc, identb[:])
            identf = const_pool.tile([128, 128], f32, tag="identf")
            make_identity(nc, identf[:])

            vp = vp_pool.tile([128, NKC * 65], bf16, tag="vp")
            nc.vector.memset(
                vp[:].rearrange("p (c e) -> p c e", e=65)[:, :, 64:65], 1.0
            )

            # ---- Q projection ----
            qprojT = proj_pool.tile([D, QSH], bf16, tag="qprojT")
            for qn in range(QSH // 512):
                ps = pj_psum.tile([D, 512], f32, tag="pj", name="pjq")
                for hc in range(HC):
                    nc.tensor.matmul(
                        ps[:],
                        w_sb["wq"][:, hc * D : (hc + 1) * D],
                        qT[:, hc * QSH + qn * 512 : hc * QSH + (qn + 1) * 512],
                        start=(hc == 0), stop=(hc == HC - 1),
                    )
                nc.vector.tensor_scalar_add(
                    qprojT[:, qn * 512 : (qn + 1) * 512], ps[:], bq_sb[:]
                )

            def kproj(rng):
                ps = pj_psum.tile([D, 512], f32, tag="pj", name=f"pjk{rng}")
                for hc in range(HC):
                    nc.tensor.matmul(
                        ps[:], w_sb["wk"][:, hc * D : (hc + 1) * D],
                        kts[rng][:, hc * 512 : (hc + 1) * 512],
                        start=(hc == 0), stop=(hc == HC - 1),
                    )
                ks = proj_pool.tile([D, 512], bf16, tag=f"ks{rng % 2}",
                                    name=f"ks{rng}")
                nc.vector.tensor_scalar_add(ks[:], ps[:], bk_sb[:])
                return ks

            def vproj_trans(rng):
                ps = pj_psum.tile([D, 512], f32, tag="pj", name=f"pjv{rng}")
                for hc in range(HC):
                    nc.tensor.matmul(
                        ps[:], w_sb["wv"][:, hc * D : (hc + 1) * D],
                        vts[rng][:, hc * 512 : (hc + 1) * 512],
                        start=(hc == 0), stop=(hc == HC - 1),
                    )
                vs = proj_pool.tile([D, 512], bf16, tag=f"vs{rng % 2}",
                                    name=f"vs{rng}")
                nc.vector.tensor_scalar_add(vs[:], ps[:], bv_sb[:])
                psv = psv_psum.tile([128, 512], bf16, tag="psv", name="psv")
                for s in range(4):
                    nc.tensor.transpose(
                        psv[:, s * 128 : s * 128 + D],
                        vs[:, s * 128 : (s + 1) * 128],
                        identb[0:D, 0:D],
                    )
                for s in range(4):
                    kc = rng * 4 + s
                    nc.vector.tensor_copy(
                        vp[:, kc * 65 : kc * 65 + 64],
                        psv[:, s * 128 : s * 128 + D],
                    )

            outT_ps = acc_psum.tile([65, QSH], f32)
            ks_tiles = {}

            def issue_sc(kc):
                """scores for key-chunk kc + exp per 512 half; returns e."""
                rng, s = kc // 4, kc % 4
                ks = ks_tiles[rng]
                e = exp_pool.tile([128, QSH], bf16, tag="exp")
                for qn in range(QSH // 512):
                    sct = sc_psum.tile([128, 512], f32, tag="sc")
                    nc.tensor.matmul(
                        sct[:],
                        ks[:, s * 128 : (s + 1) * 128],
                        qprojT[:, qn * 512 : (qn + 1) * 512],
                        start=True, stop=True,
                    )
                    nc.scalar.activation(
                        e[:, qn * 512 : (qn + 1) * 512], sct[:], AF.Exp
                    )
                return e

            def issue_av(kc, e):
                for qn in range(QSH // 512):
                    nc.tensor.matmul(
                        outT_ps[:, qn * 512 : (qn + 1) * 512],
                        vp[:, kc * 65 : (kc + 1) * 65],
                        e[:, qn * 512 : (qn + 1) * 512],
                        start=(kc == 0), stop=(kc == NKC - 1),
                        skip_group_check=True,
                    )

            # prime two ranges of K/V projection work
            ks_tiles[0] = kproj(0)
            vproj_trans(0)
            ks_tiles[1] = kproj(1)
            vproj_trans(1)
            epipe = [issue_sc(0), issue_sc(1)]
            for kc in range(NKC):
                e_cur = epipe.pop(0)
                if kc % 4 == 2 and kc // 4 + 2 < NRNG:
                    rng = kc // 4 + 2
                    ks_tiles[rng] = kproj(rng)
                    vproj_trans(rng)
                if kc + 2 < NKC:
                    epipe.append(issue_sc(kc + 2))
                issue_av(kc, e_cur)

            # ---- finalize ----
            outT_sb = fin_pool.tile([65, QSH], f32, tag="outT")
            nc.vector.tensor_copy(outT_sb[:], outT_ps[:])
            out_sb = fin_pool.tile([128, 8 * D], f32, tag="out")
            for qc in range(QSH // 128):
                ps = pj_psum.tile([128, 128], f32, tag="pj", name="pjf")
                nc.tensor.transpose(
                    ps[:, 0:65],
                    outT_sb[:, qc * 128 : (qc + 1) * 128],
                    identf[0:65, 0:65],
                )
                recip = fin_pool.tile([128, 1], f32, tag="recip")
                nc.vector.reciprocal(recip[:], ps[:, 64:65])
                nc.vector.tensor_scalar_mul(
                    out_sb[:, qc * D : (qc + 1) * D], ps[:, 0:D], recip[:]
                )
            nc.sync.dma_start(
                out=out_d.rearrange("(j p) d -> p j d", p=128),
                in_=out_sb[:].rearrange("p (j d) -> p j d", j=8),
            )

    nc.compile()
    return nc


_NC_CACHE = None


def _get_nc():
    global _NC_CACHE
    if _NC_CACHE is None:
        _NC_CACHE = build_bass()
    return _NC_CACHE


def _make_in_maps(inputs):
    import ml_dtypes

    bf = ml_dtypes.bfloat16
    # [N, L, H] -> [N, H, L] transposed bf16, contiguous
    qt = np.ascontiguousarray(
        np.asarray(inputs["query"], np.float32).astype(bf).transpose(0, 2, 1)
    )
    kt = np.ascontiguousarray(
        np.asarray(inputs["key"], np.float32).astype(bf).transpose(0, 2, 1)
    )
    vt = np.ascontiguousarray(
        np.asarray(inputs["value"], np.float32).astype(bf).transpose(0, 2, 1)
    )
    # fold the 1/sqrt(D) score scale into Wq and bq
    wq = np.ascontiguousarray(
        (np.asarray(inputs["Wq"], np.float32) / 8.0).astype(bf)
    )
    wk = np.ascontiguousarray(np.asarray(inputs["Wk"], np.float32).astype(bf))
    wv = np.ascontiguousarray(np.asarray(inputs["Wv"], np.float32).astype(bf))
    bq8 = (np.asarray(inputs["bq"], np.float32) / 8.0).reshape(D, 1)
    bk = np.asarray(inputs["bk"], np.float32).reshape(D, 1).copy()
    bv = np.asarray(inputs["bv"], np.float32).reshape(D, 1).copy()
    in_maps = []
    for c in range(NCORES):
        b, half = divmod(c, 2)
        in_maps.append(
            {
                "qT": np.ascontiguousarray(
                    qt[b, :, half * QSH : (half + 1) * QSH]
                ),
                "kT": kt[b],
                "vT": vt[b],
                "wq": wq,
                "wk": wk,
                "wv": wv,
                "bq8": bq8,
                "bk": bk,
                "bv": bv,
            }
        )
    return in_maps


def kernel(query, key, value, Wq, bq, Wk, bk, Wv, bv):
    from concourse.bass_utils import run_bass_kernel_spmd

    in_maps = _make_in_maps(
        dict(query=query, key=key, value=value, Wq=Wq, bq=bq, Wk=Wk, bk=bk,
             Wv=Wv, bv=bv)
    )
    nc = _get_nc()
    try:
        res = run_bass_kernel_spmd(nc, in_maps, list(range(NCORES)))
    except Exception:
        res = run_bass_kernel_spmd(nc, in_maps, list(range(NCORES)))
    out = np.empty((N, L, D), np.float32)
    for c in range(NCORES):
        b, half = divmod(c, 2)
        out[b, half * QSH : (half + 1) * QSH] = res.results[c]["out"]
    return out


# revision 10
# speedup vs baseline: 1.3174x; 1.1761x over previous
"""v5.5: bf16 attention on 8 trn2 NeuronCores, host-transposed inputs.

Core c handles batch c//2, query half c%2, loading the batch's full K and V.
Host pre-casts q/k/v to bf16, pre-transposes them to [H, L] layout, and
packs all three weight matrices (Wq pre-scaled by 1/sqrt(64)) into one
SBUF-layout block so every DMA moves 1KB+ descriptors. Loads go down the
two serial HWDGE FIFO queues (sync + scalar) in exact consumption order --
SWDGE stripes all queued DMAs across the 16 engines so everything would
complete late together; the HWDGE FIFOs deliver each 512-range of K/V just
in time. Projection bias adds run on the DVE; the Act engine only does exp.
Scores for chunk kc+2 issue before attnV of chunk kc so the PE pipeline
never drains. Softmax free-dim is never reduced: a ones-column in the attnV
stationary yields row-sums; normalization happens in the batched output
epilogue.
"""

import sys

if "/opt/trn_rl_repo" not in sys.path:
    sys.path.insert(0, "/opt/trn_rl_repo")

import numpy as np

N, L, H, D = 4, 2048, 1024, 64
QSH = L // 2
NCORES = 8
HC = H // 128
NRNG = L // 512
NKC = L // 128


def build_bass():
    import concourse.bass as bass
    import concourse.mybir as mybir
    from concourse import bacc
    from concourse.masks import make_identity
    from concourse.tile import TileContext

    f32 = mybir.dt.float32
    bf16 = mybir.dt.bfloat16
    AF = mybir.ActivationFunctionType

    nc = bacc.Bacc("TRN2", target_bir_lowering=False, debug=False)
    # host-transposed, bf16: qT [H, QSH], kT/vT [H, L]
    qT_d = nc.dram_tensor("qT", [H, QSH], bf16, kind="ExternalInput").ap()
    kT_d = nc.dram_tensor("kT", [H, L], bf16, kind="ExternalInput").ap()
    vT_d = nc.dram_tensor("vT", [H, L], bf16, kind="ExternalInput").ap()
    # weights pre-packed on host into SBUF layout [128, (q|k|v) x hc x D]
    w3_d = nc.dram_tensor("w3", [128, 3 * HC * D], bf16,
                          kind="ExternalInput").ap()
    b3_d = nc.dram_tensor("b3", [D, 3], f32, kind="ExternalInput").ap()
    out_d = nc.dram_tensor("out", [QSH, D], f32, kind="ExternalOutput").ap()

    with TileContext(nc) as tc:
        with (
            tc.tile_pool(name="const", bufs=1) as const_pool,
            tc.tile_pool(name="w", bufs=1) as w_pool,
            tc.tile_pool(name="qT", bufs=1) as qT_pool,
            tc.tile_pool(name="kt", bufs=4) as kt_pool,
            tc.tile_pool(name="vt", bufs=4) as vt_pool,
            tc.tile_pool(name="proj", bufs=1) as proj_pool,
            tc.tile_pool(name="vp", bufs=1) as vp_pool,
            tc.tile_pool(name="exp", bufs=4) as exp_pool,
            tc.tile_pool(name="fin", bufs=1) as fin_pool,
            tc.tile_pool(name="pj", bufs=1, space="PSUM") as pj_psum,
            tc.tile_pool(name="sc", bufs=4, space="PSUM") as sc_psum,
            tc.tile_pool(name="psv", bufs=1, space="PSUM") as psv_psum,
            tc.tile_pool(name="acc", bufs=1, space="PSUM") as acc_psum,
        ):
            # sync FIFO: weights, qTa, kT0..3   scalar FIFO: b3, qTb, vT0..3
            w3 = w_pool.tile([128, 3 * HC * D], bf16, tag="w3")
            nc.sync.dma_start(out=w3[:], in_=w3_d[:])
            b3 = const_pool.tile([D, 3], f32, tag="b3")
            nc.scalar.dma_start(out=b3[:], in_=b3_d[:])
            w_sb = {
                "wq": w3[:, 0 : HC * D],
                "wk": w3[:, HC * D : 2 * HC * D],
                "wv": w3[:, 2 * HC * D : 3 * HC * D],
            }
            bq_sb, bk_sb, bv_sb = b3[:, 0:1], b3[:, 1:2], b3[:, 2:3]

            qTh = []
            for qn, eng in ((0, nc.sync), (1, nc.scalar)):
                t = qT_pool.tile([128, HC * 512], bf16, tag=f"qT{qn}",
                                 name=f"qT{qn}")
                eng.dma_start(
                    out=t[:].rearrange("p (c l) -> p c l", c=HC),
                    in_=qT_d.rearrange("(c p) l -> p c l", p=128)[
                        :, :, qn * 512 : (qn + 1) * 512
                    ],
                )
                qTh.append(t)
            kts, vts = [], []
            for rng in range(NRNG):
                kt = kt_pool.tile([128, HC * 512], bf16, tag="kt",
                                  name=f"kT{rng}")
                nc.sync.dma_start(
                    out=kt[:].rearrange("p (c l) -> p c l", c=HC),
                    in_=kT_d.rearrange("(c p) l -> p c l", p=128)[
                        :, :, rng * 512 : (rng + 1) * 512
                    ],
                )
                kts.append(kt)
                vt = vt_pool.tile([128, HC * 512], bf16, tag="vt",
                                  name=f"vT{rng}")
                nc.scalar.dma_start(
                    out=vt[:].rearrange("p (c l) -> p c l", c=HC),
                    in_=vT_d.rearrange("(c p) l -> p c l", p=128)[
                        :, :, rng * 512 : (rng + 1) * 512
                    ],
                )
                vts.append(vt)

            identb = const_pool.tile([128, 128], bf16)
            make_identity(nc, identb[:])

            vp = vp_pool.tile([128, NKC * 65], bf16, tag="vp")
            nc.vector.memset(
                vp[:].rearrange("p (c e) -> p c e", e=65)[:, :, 64:65], 1.0
            )

            # ---- Q projection (per loaded half) ----
            qprojT = proj_pool.tile([D, QSH], bf16, tag="qprojT")
            for qn in range(2):
                ps = pj_psum.tile([D, 512], f32, tag="pj", name="pjq")
                for hc in range(HC):
                    nc.tensor.matmul(
                        ps[:],
                        w_sb["wq"][:, hc * D : (hc + 1) * D],
                        qTh[qn][:, hc * 512 : (hc + 1) * 512],
                        start=(hc == 0), stop=(hc == HC - 1),
                    )
                nc.vector.tensor_scalar_add(
                    qprojT[:, qn * 512 : (qn + 1) * 512], ps[:], bq_sb
                )

            def kproj(rng):
                ps = pj_psum.tile([D, 512], f32, tag="pj", name=f"pjk{rng}")
                for hc in range(HC):
                    nc.tensor.matmul(
                        ps[:], w_sb["wk"][:, hc * D : (hc + 1) * D],
                        kts[rng][:, hc * 512 : (hc + 1) * 512],
                        start=(hc == 0), stop=(hc == HC - 1),
                    )
                ks = proj_pool.tile([D, 512], bf16, tag=f"ks{rng % 2}",
                                    name=f"ks{rng}")
                nc.vector.tensor_scalar_add(ks[:], ps[:], bk_sb)
                return ks

            def vproj_trans(rng):
                ps = pj_psum.tile([D, 512], f32, tag="pj", name=f"pjv{rng}")
                for hc in range(HC):
                    nc.tensor.matmul(
                        ps[:], w_sb["wv"][:, hc * D : (hc + 1) * D],
                        vts[rng][:, hc * 512 : (hc + 1) * 512],
                        start=(hc == 0), stop=(hc == HC - 1),
                    )
                vs = proj_pool.tile([D, 512], bf16, tag=f"vs{rng % 2}",
                                    name=f"vs{rng}")
                nc.vector.tensor_scalar_add(vs[:], ps[:], bv_sb)
                psv = psv_psum.tile([128, 512], bf16, tag="psv", name="psv")
                for s in range(4):
                    nc.tensor.transpose(
                        psv[:, s * 128 : s * 128 + D],
                        vs[:, s * 128 : (s + 1) * 128],
                        identb[0:D, 0:D],
                    )
                for s in range(4):
                    kc = rng * 4 + s
                    nc.vector.tensor_copy(
                        vp[:, kc * 65 : kc * 65 + 64],
                        psv[:, s * 128 : s * 128 + D],
                    )

            outT_ps = acc_psum.tile([65, QSH], f32)
            ks_tiles = {}

            def issue_sc(kc):
                """scores for key-chunk kc + exp per 512 half; returns e."""
                rng, s = kc // 4, kc % 4
                ks = ks_tiles[rng]
                e = exp_pool.tile([128, QSH], bf16, tag="exp")
                for qn in range(2):
                    sct = sc_psum.tile([128, 512], f32, tag="sc")
                    nc.tensor.matmul(
                        sct[:],
                        ks[:, s * 128 : (s + 1) * 128],
                        qprojT[:, qn * 512 : (qn + 1) * 512],
                        start=True, stop=True,
                    )
                    nc.scalar.activation(
                        e[:, qn * 512 : (qn + 1) * 512], sct[:], AF.Exp
                    )
                return e

            def issue_av(kc, e):
                for qn in range(2):
                    nc.tensor.matmul(
                        outT_ps[:, qn * 512 : (qn + 1) * 512],
                        vp[:, kc * 65 : (kc + 1) * 65],
                        e[:, qn * 512 : (qn + 1) * 512],
                        start=(kc == 0), stop=(kc == NKC - 1),
                        skip_group_check=True,
                    )

            ks_tiles[0] = kproj(0)
            vproj_trans(0)
            ks_tiles[1] = kproj(1)
            vproj_trans(1)
            epipe = [issue_sc(0), issue_sc(1)]
            for kc in range(NKC):
                e_cur = epipe.pop(0)
                if kc % 4 == 2 and kc // 4 + 2 < NRNG:
                    rng = kc // 4 + 2
                    ks_tiles[rng] = kproj(rng)
                    vproj_trans(rng)
                if kc + 2 < NKC:
                    epipe.append(issue_sc(kc + 2))
                issue_av(kc, e_cur)

            # ---- finalize: batched transposes, strided reciprocal ----
            outT_sb = fin_pool.tile([65, QSH], f32, tag="outT")
            nc.vector.tensor_copy(outT_sb[:], outT_ps[:])
            idento = fin_pool.tile([65, 65], f32, tag="identf")
            make_identity(nc, idento[:])
            out_sb = fin_pool.tile([128, 8 * D], f32, tag="out")
            for half in range(2):
                ft = sc_psum.tile([128, 512], f32, tag="sc", name=f"fin{half}")
                for j in range(4):
                    qc = half * 4 + j
                    nc.tensor.transpose(
                        ft[:, j * 128 : j * 128 + 65],
                        outT_sb[:, qc * 128 : (qc + 1) * 128],
                        idento[:],
                    )
                recip = fin_pool.tile([128, 4], f32, tag=f"recip{half}",
                                      name=f"recip{half}")
                nc.vector.reciprocal(
                    recip[:],
                    ft[:].rearrange("p (j c) -> p j c", c=128)[:, :, 64:65],
                )
                for j in range(4):
                    qc = half * 4 + j
                    nc.vector.tensor_scalar_mul(
                        out_sb[:, qc * D : (qc + 1) * D],
                        ft[:, j * 128 : j * 128 + D],
                        recip[:, j : j + 1],
                    )
            nc.sync.dma_start(
                out=out_d.rearrange("(j p) d -> p j d", p=128),
                in_=out_sb[:].rearrange("p (j d) -> p j d", j=8),
            )

    nc.compile()
    return nc


_NC_CACHE = None


def _get_nc():
    global _NC_CACHE
    if _NC_CACHE is None:
        _NC_CACHE = build_bass()
    return _NC_CACHE


def _make_in_maps(inputs):
    import ml_dtypes

    bf = ml_dtypes.bfloat16
    # [N, L, H] -> [N, H, L] transposed bf16, contiguous
    qt = np.ascontiguousarray(
        np.asarray(inputs["query"], np.float32).astype(bf).transpose(0, 2, 1)
    )
    kt = np.ascontiguousarray(
        np.asarray(inputs["key"], np.float32).astype(bf).transpose(0, 2, 1)
    )
    vt = np.ascontiguousarray(
        np.asarray(inputs["value"], np.float32).astype(bf).transpose(0, 2, 1)
    )
    # pack weights into SBUF layout [128, (q|k|v) x hc x D]; fold 1/sqrt(D)
    # score scale into Wq/bq
    w3 = np.empty((128, 3 * HC * D), np.float32)
    for t, (wname, scale) in enumerate(
        (("Wq", 0.125), ("Wk", 1.0), ("Wv", 1.0))
    ):
        w = np.asarray(inputs[wname], np.float32) * scale  # [H, D]
        # w3[p, (t*HC + c)*D + d] = w[c*128 + p, d]
        w3[:, t * HC * D : (t + 1) * HC * D] = (
            w.reshape(HC, 128, D).transpose(1, 0, 2).reshape(128, HC * D)
        )
    w3 = np.ascontiguousarray(w3.astype(bf))
    b3 = np.stack(
        [
            np.asarray(inputs["bq"], np.float32) / 8.0,
            np.asarray(inputs["bk"], np.float32),
            np.asarray(inputs["bv"], np.float32),
        ],
        axis=1,
    )
    b3 = np.ascontiguousarray(b3)

    in_maps = []
    for c in range(NCORES):
        b, half = divmod(c, 2)
        in_maps.append(
            {
                "qT": np.ascontiguousarray(
                    qt[b, :, half * QSH : (half + 1) * QSH]
                ),
                "kT": kt[b],
                "vT": vt[b],
                "w3": w3,
                "b3": b3,
            }
        )
    return in_maps


def kernel(query, key, value, Wq, bq, Wk, bk, Wv, bv):
    from concourse.bass_utils import run_bass_kernel_spmd

    in_maps = _make_in_maps(
        dict(query=query, key=key, value=value, Wq=Wq, bq=bq, Wk=Wk, bk=bk,
             Wv=Wv, bv=bv)
    )
    nc = _get_nc()
    try:
        res = run_bass_kernel_spmd(nc, in_maps, list(range(NCORES)))
    except Exception:
        res = run_bass_kernel_spmd(nc, in_maps, list(range(NCORES)))
    out = np.empty((N, L, D), np.float32)
    for c in range(NCORES):
        b, half = divmod(c, 2)
        out[b, half * QSH : (half + 1) * QSH] = res.results[c]["out"]
    return out
